# revision 12
# baseline (speedup 1.0000x reference)
"""CategoryConsistencyLoss kernel for 8 trn2 NeuronCores.

loss = mean_i clip(||x_i - w_{labels_i}||^2, 1e-12, 1e12)

The reference materializes the full [N, C] squared-distance matrix and
gathers the label-indexed entries. Two observations collapse the work:

1. Only the N label-indexed entries matter -> O(N*D), not O(N*C*D).
2. The output is a SCALAR mean, and the clip never binds for data in this
   regime (row distances concentrate around 2*D = 4096, far inside
   [1e-12, 1e12]), so per-row distances are never needed:

       loss * N = sum(x*x) + sum_c cnt_c*||w_c||^2 - 2*sum_c <S_c, w_c>

   where S_c = sum of x rows with label c. S is computed on the
   TensorEngine as sel^T @ x (sel built on-device from label codes), and
   the cnt*||w||^2 term is folded into the same PSUM accumulation via one
   extra matmul with lhsT = diag(-0.5*cnt):

       loss * N = sum(x*x) - 2*sum<S', wt>,  S' = sel^T @ x - 0.5*cnt (.) wt

Rows are sorted by label on the host so each 128-row tile spans <=16
distinct classes; per-tile class windows pack into G=2 combined [128, D]
weight tables (duplicate classes across tiles are harmless: the per-slot
dot/cnt sums still total correctly).

Performance structure (per core):
- x streams as fp8-e4m3 (4.2 MB vs 16.8 f32; quantization bias ~4e-4,
  gate is 2e-2), host-packed into three contiguous column-split buffers
  (1024/512/512 per tile, concatenated across tiles) so the sum(x*x)
  engines process multi-tile contiguous spans with few instructions,
  while the per-tile matmuls read 512-column slices (subtile deps).
- sum(x*x) is split between ACT (square+accumulate) and DVE
  (scalar_tensor_tensor fused square+reduce); the split point is a knob.
- All DMA triggers issue from the GPSIMD queue (25ns each vs 600ns on
  the sync queue, which otherwise serializes the stream front).
- All 16 sel one-hot matrices build in one DVE is_equal op.

Sharding: data-parallel over N across the 8 cores. Each core returns
[128, 3T+G] partial sums; the host does the final (tiny) reduction.
"""

import numpy as np
import ml_dtypes

import concourse.bacc as bacc
import concourse.mybir as mybir
import concourse.tile as tile
from concourse import bass_utils

N, C, D = 16384, 1000, 2048
N_CORES = 8
N_LOC = N // N_CORES  # 2048 rows per core
P = 128               # SBUF partitions
T = N_LOC // P        # 16 tiles per core

# x column-split buffer widths (multiples of 512 so matmul chunks stay
# within PSUM banks)
W0, W1, W2 = 1024, 512, 512
# ACT takes all of buf0 plus the first ACT_B1_TILES tiles of buf1; DVE
# takes the rest of buf1 and all of buf2.
ACT_B1_TILES = 8
# op chunking (tiles per instruction) for the xsq spans
XCHUNK = [4, 4, 4, 2, 1, 1]

_nc_cache = {}
LAST_RESULTS = None  # BassKernelResults of the most recent run (for profiling)

F8 = mybir.dt.float8e4
F8_NP = ml_dtypes.float8_e4m3


def _spans(n_tiles):
    """Split n_tiles into op spans per XCHUNK (front-loaded big chunks)."""
    out, t = [], 0
    for c in XCHUNK:
        if t >= n_tiles:
            break
        c = min(c, n_tiles - t)
        out.append((t, t + c))
        t += c
    while t < n_tiles:
        out.append((t, t + 1))
        t += 1
    return out


def _build(u_rows):
    tpg = P // u_rows          # tiles per group
    G = -(-T // tpg)           # number of groups
    nc = bacc.Bacc("TRN2", target_bir_lowering=False, debug=False)
    f32 = mybir.dt.float32
    bf16 = mybir.dt.bfloat16
    b0_d = nc.dram_tensor("b0", [P, T * W0], F8, kind="ExternalInput")
    b1_d = nc.dram_tensor("b1", [P, T * W1], F8, kind="ExternalInput")
    b2_d = nc.dram_tensor("b2", [P, T * W2], F8, kind="ExternalInput")
    wt_d = nc.dram_tensor("wt", [G * P, D], bf16, kind="ExternalInput")
    e2_d = nc.dram_tensor("e2", [P, T], f32, kind="ExternalInput")
    dc_d = nc.dram_tensor("dc", [G * P, P], bf16, kind="ExternalInput")
    n_xsq = (
        len(_spans(T)) * 2
        + len(_spans(ACT_B1_TILES))
        + len(_spans(T - ACT_B1_TILES))
    )
    out_d = nc.dram_tensor("acc", [P, n_xsq + G], f32, kind="ExternalOutput")

    with tile.TileContext(nc) as tc:
        with (
            tc.tile_pool(name="big", bufs=1) as bpool,
            tc.tile_pool(name="psum", bufs=1, space="PSUM") as pspool,
            tc.tile_pool(name="small", bufs=1) as spool,
        ):
            b0 = bpool.tile([P, T * W0], F8)
            b1 = bpool.tile([P, T * W1], F8)
            b2 = bpool.tile([P, T * W2], F8)
            e2_sb = spool.tile([P, T], f32)
            sels = spool.tile([P, T * P], F8)
            iota_sb = spool.tile([P, T * P], f32)
            acc = spool.tile([P, n_xsq + G], f32)
            scr_a = spool.tile([P, 4 * W0 + 1], bf16)
            scr_d = spool.tile([P, 4 * W2 + 1], bf16)
            scr_big = spool.tile([P, D], bf16)
            wt_sb = [None] * G
            dc_sb = [None] * G
            S = [None] * G

            # iota repeating 0..P-1 per tile window, f32 (is_equal wants f32)
            nc.gpsimd.iota(
                iota_sb[:],
                pattern=[[0, T], [1, P]],
                base=0,
                channel_multiplier=0,
                allow_small_or_imprecise_dtypes=True,
            )

            # DMA plan: everything triggers from the (otherwise idle) GPSIMD
            # queue where a trigger costs ~25ns, in consumption order.
            dchunks = _spans(T)

            def dma_xchunk(ci):
                t0, t1 = dchunks[ci]
                nc.gpsimd.dma_start(
                    out=b0[:, t0 * W0 : t1 * W0],
                    in_=b0_d.ap()[:, t0 * W0 : t1 * W0],
                )
                nc.gpsimd.dma_start(
                    out=b1[:, t0 * W1 : t1 * W1],
                    in_=b1_d.ap()[:, t0 * W1 : t1 * W1],
                )
                nc.gpsimd.dma_start(
                    out=b2[:, t0 * W2 : t1 * W2],
                    in_=b2_d.ap()[:, t0 * W2 : t1 * W2],
                )

            def load_group(g):
                wt_sb[g] = spool.tile([P, D], bf16, tag=f"wt{g}", name=f"wt{g}")
                nc.gpsimd.dma_start(
                    out=wt_sb[g][:], in_=wt_d.ap()[g * P : (g + 1) * P, :]
                )
                dc_sb[g] = spool.tile([P, P], bf16, tag=f"dc{g}", name=f"dc{g}")
                nc.gpsimd.dma_start(
                    out=dc_sb[g][:], in_=dc_d.ap()[g * P : (g + 1) * P, :]
                )

            dma_xchunk(0)
            nc.gpsimd.dma_start(out=e2_sb[:], in_=e2_d.ap()[:])
            dma_xchunk(1)
            load_group(0)
            dma_xchunk(2)
            if G > 1:
                load_group(1)
            for ci in range(3, len(dchunks)):
                dma_xchunk(ci)
            for g in range(2, G):
                load_group(g)  # u_rows > 16 generality; unused when G == 2

            # sel[row, t*P + slot] = (e2[row, t] == slot): all 16 one-hot
            # selection matrices in one is_equal op, exact 0/1 in fp8.
            nc.vector.tensor_tensor(
                out=sels[:].rearrange("p (t s) -> p t s", s=P),
                in0=iota_sb[:].rearrange("p (t s) -> p t s", s=P),
                in1=e2_sb[:].unsqueeze(2).to_broadcast([P, T, P]),
                op=mybir.AluOpType.is_equal,
            )

            # Per-tile matmuls: S'[g] accumulates sel^T @ x over the group's
            # tiles, then -0.5*cnt (.) wt via the diag matmul, then the DVE
            # drains <S', wt>.
            for t in range(T):
                g = t // tpg
                if t % tpg == 0:
                    S[g] = pspool.tile(
                        [P, D], f32, tag=f"S{g % 2}", name=f"S{g}"
                    )
                start = t % tpg == 0
                sel_t = sels[:, t * P : (t + 1) * P]
                for q, (buf, w, qo) in enumerate(
                    [(b0, W0, 0), (b0, W0, 512), (b1, W1, 0), (b2, W2, 0)]
                ):
                    nc.tensor.matmul(
                        out=S[g][:, q * 512 : (q + 1) * 512],
                        lhsT=sel_t,
                        rhs=buf[:, t * w + qo : t * w + qo + 512],
                        start=start,
                        stop=False,
                    )
                if t % tpg == tpg - 1 or t == T - 1:
                    for q in range(4):
                        nc.tensor.matmul(
                            out=S[g][:, q * 512 : (q + 1) * 512],
                            lhsT=dc_sb[g][:],
                            rhs=wt_sb[g][:, q * 512 : (q + 1) * 512],
                            start=False,
                            stop=True,
                        )
                    nc.vector.scalar_tensor_tensor(
                        out=scr_big[:],
                        in0=S[g][:],
                        scalar=1.0,
                        in1=wt_sb[g][:],
                        op0=mybir.AluOpType.mult,
                        op1=mybir.AluOpType.mult,
                        accum_out=acc[:, n_xsq + g : n_xsq + g + 1],
                    )

            # sum(x*x): ACT covers buf0 + the first ACT_B1_TILES tiles of
            # buf1; DVE covers the rest. Ops span multiple tiles (chunked
            # per XCHUNK) to amortize per-instruction overheads.
            ai = 0
            for t0, t1 in _spans(T):  # ACT on buf0
                nc.scalar.activation(
                    out=scr_a[:, 0 : (t1 - t0) * W0],
                    in_=b0[:, t0 * W0 : t1 * W0],
                    func=mybir.ActivationFunctionType.Square,
                    accum_out=acc[:, ai : ai + 1],
                )
                ai += 1
            for t0, t1 in _spans(ACT_B1_TILES):  # ACT share of buf1
                nc.scalar.activation(
                    out=scr_a[:, 0 : (t1 - t0) * W1],
                    in_=b1[:, t0 * W1 : t1 * W1],
                    func=mybir.ActivationFunctionType.Square,
                    accum_out=acc[:, ai : ai + 1],
                )
                ai += 1

            def dve_sq(src, c0, c1):
                nonlocal ai
                nc.vector.scalar_tensor_tensor(
                    out=scr_d[:, 0 : c1 - c0],
                    in0=src[:, c0:c1],
                    scalar=1.0,
                    in1=src[:, c0:c1],
                    op0=mybir.AluOpType.mult,
                    op1=mybir.AluOpType.mult,
                    accum_out=acc[:, ai : ai + 1],
                )
                ai += 1

            for t0, t1 in _spans(T - ACT_B1_TILES):  # DVE rest of buf1
                dve_sq(b1, (ACT_B1_TILES + t0) * W1, (ACT_B1_TILES + t1) * W1)
            for t0, t1 in _spans(T):  # DVE all of buf2
                dve_sq(b2, t0 * W2, t1 * W2)
            assert ai == n_xsq

            nc.sync.dma_start(out=out_d.ap()[:], in_=acc[:])
    nc.compile()
    return nc, G, n_xsq


def _prep_core(ls_c, tile_u, w, u_rows, tpg, G):
    """Per-core host-side packing: weight tables, codes, count diagonals."""
    wt = np.zeros((G * P, D), dtype=np.float32)
    e2 = np.zeros((P, T), dtype=np.float32)
    cnt = np.zeros((G, P), dtype=np.float64)
    for t in range(T):
        gu = tile_u[t]
        g = t // tpg
        slot = (t % tpg) * u_rows
        wt[g * P + slot : g * P + slot + len(gu)] = w[gu]
        codes = slot + np.searchsorted(gu, ls_c[t * P : (t + 1) * P])
        e2[:, t] = codes
        cnt[g] += np.bincount(codes.astype(np.int64), minlength=P)
    dc = np.zeros((G * P, P), dtype=np.float32)
    for g in range(G):
        dc[g * P : (g + 1) * P][np.arange(P), np.arange(P)] = -0.5 * cnt[g]
    return {
        "wt": wt.astype(ml_dtypes.bfloat16),
        "e2": e2,
        "dc": dc.astype(ml_dtypes.bfloat16),
    }


def _pack_x(x_core_f8):
    """Split a core's [N_LOC, D] fp8 rows into the three column-split
    buffers: b_k[p, t*Wk + j] = x[t*128 + p, off_k + j]."""
    xs = x_core_f8.reshape(T, P, D)
    b0 = np.ascontiguousarray(xs[:, :, 0:W0].transpose(1, 0, 2)).reshape(P, T * W0)
    b1 = np.ascontiguousarray(xs[:, :, W0 : W0 + W1].transpose(1, 0, 2)).reshape(
        P, T * W1
    )
    b2 = np.ascontiguousarray(xs[:, :, W0 + W1 : D].transpose(1, 0, 2)).reshape(
        P, T * W2
    )
    return b0, b1, b2


def kernel(x, labels, weightcenters):
    global LAST_RESULTS
    x = np.asarray(x, dtype=np.float32)
    labels = np.asarray(labels, dtype=np.int32)
    w = np.asarray(weightcenters, dtype=np.float32)

    # Global sort by label so each 128-row tile spans few classes.
    gorder = np.argsort(labels, kind="stable")
    x_sorted = np.ascontiguousarray(x[gorder]).astype(F8_NP)
    l_sorted = labels[gorder]

    shard_labels = [l_sorted[c * N_LOC : (c + 1) * N_LOC] for c in range(N_CORES)]
    tile_u = [
        [np.unique(ls[t * P : (t + 1) * P]) for t in range(T)]
        for ls in shard_labels
    ]
    u_max = max(len(u) for us in tile_u for u in us)
    u_rows = min(P, -(-u_max // 8) * 8)
    while P % u_rows:
        u_rows += 8
    tpg = P // u_rows

    if u_rows not in _nc_cache:
        _nc_cache[u_rows] = _build(u_rows)
    nc, G, n_xsq = _nc_cache[u_rows]

    in_maps = []
    for c in range(N_CORES):
        m = _prep_core(shard_labels[c], tile_u[c], w, u_rows, tpg, G)
        m["b0"], m["b1"], m["b2"] = _pack_x(x_sorted[c * N_LOC : (c + 1) * N_LOC])
        in_maps.append(m)

    # The axon-tunneled device occasionally starts in a wedged state left by
    # a previous process and recovers after a short wait; retry around it.
    last_exc = None
    for attempt in range(5):
        try:
            res = bass_utils.run_bass_kernel_spmd(
                nc, in_maps, core_ids=list(range(N_CORES))
            )
            break
        except Exception as exc:  # noqa: BLE001 — device transients
            last_exc = exc
            import time as _time

            _time.sleep(20 * (attempt + 1))
    else:
        raise last_exc
    LAST_RESULTS = res

    total = 0.0
    for c in range(N_CORES):
        acc = res.results[c]["acc"].astype(np.float64)
        total += acc[:, 0:n_xsq].sum() - 2.0 * acc[:, n_xsq : n_xsq + G].sum()
    return np.float32(total / N)


# revision 13
# speedup vs baseline: 1.0889x; 1.0889x over previous
"""CategoryConsistencyLoss kernel for 8 trn2 NeuronCores.

loss = mean_i clip(||x_i - w_{labels_i}||^2, 1e-12, 1e12)

The reference materializes the full [N, C] squared-distance matrix and
gathers the label-indexed entries. Two observations collapse the work:

1. Only the N label-indexed entries matter -> O(N*D), not O(N*C*D).
2. The output is a SCALAR mean, and the clip never binds for data in this
   regime (row distances concentrate around 2*D = 4096, far inside
   [1e-12, 1e12]), so per-row distances are never needed:

       loss * N = sum(x*x) + sum_c cnt_c*||w_c||^2 - 2*sum_c <S_c, w_c>

   where S_c = sum of x rows with label c. S is computed on the
   TensorEngine as sel^T @ x (sel built on-device from label codes), and
   the cnt*||w||^2 term is folded into the same PSUM accumulation via one
   extra matmul with lhsT = diag(-0.5*cnt):

       loss * N = sum(x*x) - 2*sum<S', wt>,  S' = sel^T @ x - 0.5*cnt (.) wt

Rows are sorted by label on the host so each 128-row tile spans <=16
distinct classes; per-tile class windows pack into G=2 combined [128, D]
weight tables (duplicate classes across tiles are harmless: the per-slot
dot/cnt sums still total correctly).

Performance structure (per core):
- x streams as fp8-e4m3 (4.2 MB vs 16.8 f32; quantization bias ~4e-4,
  gate is 2e-2), host-packed into three contiguous column-split buffers
  (1024/512/512 per tile, concatenated across tiles) so the sum(x*x)
  engines process multi-tile contiguous spans with few instructions,
  while the per-tile matmuls read 512-column slices (subtile deps).
- sum(x*x) is split between ACT (square+accumulate) and DVE
  (scalar_tensor_tensor fused square+reduce); the split point is a knob.
- All DMA triggers issue from the GPSIMD queue (25ns each vs 600ns on
  the sync queue, which otherwise serializes the stream front).
- All 16 sel one-hot matrices build in one DVE is_equal op.

Sharding: data-parallel over N across the 8 cores. Each core returns
[128, 3T+G] partial sums; the host does the final (tiny) reduction.
"""

import numpy as np
import ml_dtypes

import concourse.bacc as bacc
import concourse.mybir as mybir
import concourse.tile as tile
from concourse import bass_utils

N, C, D = 16384, 1000, 2048
N_CORES = 8
N_LOC = N // N_CORES  # 2048 rows per core
P = 128               # SBUF partitions
T = N_LOC // P        # 16 tiles per core

# x column-split buffer widths (multiples of 512 so matmul chunks stay
# within PSUM banks)
W0, W1, W2 = 1024, 512, 512
# ACT takes all of buf0 plus the first ACT_B1_TILES tiles of buf1; DVE
# takes the rest of buf1 and all of buf2.
ACT_B1_TILES = 9
# instruction chunking (tiles per op): front ops small for a fast start,
# tail ops small to shrink the pipeline tail
ACT0_CHUNK = [2, 2, 4, 4, 2, 1, 1]
XCHUNK = [4, 4, 4, 2, 1, 1]
# DMA transfer chunking per buffer (tiles per dma_start)
B0_DMA = [1, 1, 2, 4, 4, 4]
BX_DMA = [4, 4, 4, 4]

_nc_cache = {}
LAST_RESULTS = None  # BassKernelResults of the most recent run (for profiling)

F8 = mybir.dt.float8e4
F8_NP = ml_dtypes.float8_e4m3


def _spans(n_tiles, chunks):
    """Split n_tiles into op spans (front-loaded big chunks)."""
    out, t = [], 0
    for c in chunks:
        if t >= n_tiles:
            break
        c = min(c, n_tiles - t)
        out.append((t, t + c))
        t += c
    while t < n_tiles:
        out.append((t, t + 1))
        t += 1
    return out


def _build(u_rows):
    tpg = P // u_rows          # tiles per group
    G = -(-T // tpg)           # number of groups
    nc = bacc.Bacc("TRN2", target_bir_lowering=False, debug=False)
    f32 = mybir.dt.float32
    bf16 = mybir.dt.bfloat16
    b0_d = nc.dram_tensor("b0", [P, T * W0], F8, kind="ExternalInput")
    b1_d = nc.dram_tensor("b1", [P, T * W1], F8, kind="ExternalInput")
    b2_d = nc.dram_tensor("b2", [P, T * W2], F8, kind="ExternalInput")
    wt_d = nc.dram_tensor("wt", [G * P, D], bf16, kind="ExternalInput")
    e2_d = nc.dram_tensor("e2", [P, T], f32, kind="ExternalInput")
    dc_d = nc.dram_tensor("dc", [G * P, P], bf16, kind="ExternalInput")
    n_xsq = (
        len(_spans(T, ACT0_CHUNK))
        + len(_spans(ACT_B1_TILES, XCHUNK))
        + len(_spans(T - ACT_B1_TILES, XCHUNK))
        + len(_spans(T, XCHUNK))
    )
    out_d = nc.dram_tensor("acc", [P, n_xsq + G], f32, kind="ExternalOutput")

    with tile.TileContext(nc) as tc:
        with (
            tc.tile_pool(name="big", bufs=1) as bpool,
            tc.tile_pool(name="psum", bufs=1, space="PSUM") as pspool,
            tc.tile_pool(name="small", bufs=1) as spool,
        ):
            b0 = bpool.tile([P, T * W0], F8)
            b1 = bpool.tile([P, T * W1], F8)
            b2 = bpool.tile([P, T * W2], F8)
            e2_sb = spool.tile([P, T], f32)
            sels = spool.tile([P, T * P], F8)
            iota_sb = spool.tile([P, T * P], f32)
            acc = spool.tile([P, n_xsq + G], f32)
            scr_a = spool.tile([P, 4 * W0 + 1], bf16)
            scr_d = spool.tile([P, 4 * W2 + 1], bf16)
            scr_big = spool.tile([P, D], bf16)
            wt_sb = [None] * G
            dc_sb = [None] * G
            S = [None] * G

            # iota repeating 0..P-1 per tile window, f32 (is_equal wants f32)
            nc.gpsimd.iota(
                iota_sb[:],
                pattern=[[0, T], [1, P]],
                base=0,
                channel_multiplier=0,
                allow_small_or_imprecise_dtypes=True,
            )

            # DMA plan: triggers ride the sync queue (~0.6us each, order
            # preserved), interleaved across the three buffers in
            # consumption order; front chunks are small so the first
            # compute ops start early.
            b0c = _spans(T, B0_DMA)
            bxc = _spans(T, BX_DMA)

            def dma_b(buf, buf_d, w, t0, t1):
                nc.sync.dma_start(
                    out=buf[:, t0 * w : t1 * w], in_=buf_d.ap()[:, t0 * w : t1 * w]
                )

            def load_group(g):
                wt_sb[g] = spool.tile([P, D], bf16, tag=f"wt{g}", name=f"wt{g}")
                nc.sync.dma_start(
                    out=wt_sb[g][:], in_=wt_d.ap()[g * P : (g + 1) * P, :]
                )
                dc_sb[g] = spool.tile([P, P], bf16, tag=f"dc{g}", name=f"dc{g}")
                nc.sync.dma_start(
                    out=dc_sb[g][:], in_=dc_d.ap()[g * P : (g + 1) * P, :]
                )

            dma_b(b0, b0_d, W0, *b0c[0])
            nc.sync.dma_start(out=e2_sb[:], in_=e2_d.ap()[:])
            dma_b(b0, b0_d, W0, *b0c[1])
            dma_b(b1, b1_d, W1, *bxc[0])
            dma_b(b2, b2_d, W2, *bxc[0])
            dma_b(b0, b0_d, W0, *b0c[2])
            dma_b(b1, b1_d, W1, *bxc[1])
            dma_b(b2, b2_d, W2, *bxc[1])
            dma_b(b0, b0_d, W0, *b0c[3])
            load_group(0)
            dma_b(b1, b1_d, W1, *bxc[2])
            dma_b(b2, b2_d, W2, *bxc[2])
            dma_b(b0, b0_d, W0, *b0c[4])
            if G > 1:
                load_group(1)
            dma_b(b1, b1_d, W1, *bxc[3])
            dma_b(b2, b2_d, W2, *bxc[3])
            dma_b(b0, b0_d, W0, *b0c[5])
            for g in range(2, G):
                load_group(g)  # u_rows > 16 generality; unused when G == 2

            # sel[row, t*P + slot] = (e2[row, t] == slot): all 16 one-hot
            # selection matrices in one is_equal op, exact 0/1 in fp8.
            nc.vector.tensor_tensor(
                out=sels[:].rearrange("p (t s) -> p t s", s=P),
                in0=iota_sb[:].rearrange("p (t s) -> p t s", s=P),
                in1=e2_sb[:].unsqueeze(2).to_broadcast([P, T, P]),
                op=mybir.AluOpType.is_equal,
            )

            # Per-tile matmuls: S'[g] accumulates sel^T @ x over the group's
            # tiles, then -0.5*cnt (.) wt via the diag matmul, then the DVE
            # drains <S', wt>.
            for t in range(T):
                g = t // tpg
                if t % tpg == 0:
                    S[g] = pspool.tile(
                        [P, D], f32, tag=f"S{g % 2}", name=f"S{g}"
                    )
                start = t % tpg == 0
                sel_t = sels[:, t * P : (t + 1) * P]
                for q, (buf, w, qo) in enumerate(
                    [(b0, W0, 0), (b0, W0, 512), (b1, W1, 0), (b2, W2, 0)]
                ):
                    nc.tensor.matmul(
                        out=S[g][:, q * 512 : (q + 1) * 512],
                        lhsT=sel_t,
                        rhs=buf[:, t * w + qo : t * w + qo + 512],
                        start=start,
                        stop=False,
                    )
                if t % tpg == tpg - 1 or t == T - 1:
                    for q in range(4):
                        nc.tensor.matmul(
                            out=S[g][:, q * 512 : (q + 1) * 512],
                            lhsT=dc_sb[g][:],
                            rhs=wt_sb[g][:, q * 512 : (q + 1) * 512],
                            start=False,
                            stop=True,
                        )
                    nc.vector.scalar_tensor_tensor(
                        out=scr_big[:],
                        in0=S[g][:],
                        scalar=1.0,
                        in1=wt_sb[g][:],
                        op0=mybir.AluOpType.mult,
                        op1=mybir.AluOpType.mult,
                        accum_out=acc[:, n_xsq + g : n_xsq + g + 1],
                    )

            # sum(x*x): ACT covers buf0 + the first ACT_B1_TILES tiles of
            # buf1; DVE covers the rest. Ops span multiple tiles (chunked
            # per XCHUNK) to amortize per-instruction overheads.
            ai = 0
            for t0, t1 in _spans(T, ACT0_CHUNK):  # ACT on buf0
                nc.scalar.activation(
                    out=scr_a[:, 0 : (t1 - t0) * W0],
                    in_=b0[:, t0 * W0 : t1 * W0],
                    func=mybir.ActivationFunctionType.Square,
                    accum_out=acc[:, ai : ai + 1],
                )
                ai += 1
            for t0, t1 in _spans(ACT_B1_TILES, XCHUNK):  # ACT share of buf1
                nc.scalar.activation(
                    out=scr_a[:, 0 : (t1 - t0) * W1],
                    in_=b1[:, t0 * W1 : t1 * W1],
                    func=mybir.ActivationFunctionType.Square,
                    accum_out=acc[:, ai : ai + 1],
                )
                ai += 1

            def dve_sq(src, c0, c1):
                nonlocal ai
                nc.vector.scalar_tensor_tensor(
                    out=scr_d[:, 0 : c1 - c0],
                    in0=src[:, c0:c1],
                    scalar=1.0,
                    in1=src[:, c0:c1],
                    op0=mybir.AluOpType.mult,
                    op1=mybir.AluOpType.mult,
                    accum_out=acc[:, ai : ai + 1],
                )
                ai += 1

            for t0, t1 in _spans(T - ACT_B1_TILES, XCHUNK):  # DVE rest of buf1
                dve_sq(b1, (ACT_B1_TILES + t0) * W1, (ACT_B1_TILES + t1) * W1)
            for t0, t1 in _spans(T, XCHUNK):  # DVE all of buf2
                dve_sq(b2, t0 * W2, t1 * W2)
            assert ai == n_xsq

            nc.sync.dma_start(out=out_d.ap()[:], in_=acc[:])
    nc.compile()
    return nc, G, n_xsq


def _prep_core(ls_c, tile_u, w, u_rows, tpg, G):
    """Per-core host-side packing: weight tables, codes, count diagonals."""
    wt = np.zeros((G * P, D), dtype=np.float32)
    e2 = np.zeros((P, T), dtype=np.float32)
    cnt = np.zeros((G, P), dtype=np.float64)
    for t in range(T):
        gu = tile_u[t]
        g = t // tpg
        slot = (t % tpg) * u_rows
        wt[g * P + slot : g * P + slot + len(gu)] = w[gu]
        codes = slot + np.searchsorted(gu, ls_c[t * P : (t + 1) * P])
        e2[:, t] = codes
        cnt[g] += np.bincount(codes.astype(np.int64), minlength=P)
    dc = np.zeros((G * P, P), dtype=np.float32)
    for g in range(G):
        dc[g * P : (g + 1) * P][np.arange(P), np.arange(P)] = -0.5 * cnt[g]
    return {
        "wt": wt.astype(ml_dtypes.bfloat16),
        "e2": e2,
        "dc": dc.astype(ml_dtypes.bfloat16),
    }


def _pack_x(x_core_f8):
    """Split a core's [N_LOC, D] fp8 rows into the three column-split
    buffers: b_k[p, t*Wk + j] = x[t*128 + p, off_k + j]."""
    xs = x_core_f8.reshape(T, P, D)
    b0 = np.ascontiguousarray(xs[:, :, 0:W0].transpose(1, 0, 2)).reshape(P, T * W0)
    b1 = np.ascontiguousarray(xs[:, :, W0 : W0 + W1].transpose(1, 0, 2)).reshape(
        P, T * W1
    )
    b2 = np.ascontiguousarray(xs[:, :, W0 + W1 : D].transpose(1, 0, 2)).reshape(
        P, T * W2
    )
    return b0, b1, b2


def kernel(x, labels, weightcenters):
    global LAST_RESULTS
    x = np.asarray(x, dtype=np.float32)
    labels = np.asarray(labels, dtype=np.int32)
    w = np.asarray(weightcenters, dtype=np.float32)

    # Global sort by label so each 128-row tile spans few classes.
    gorder = np.argsort(labels, kind="stable")
    x_sorted = np.ascontiguousarray(x[gorder]).astype(F8_NP)
    l_sorted = labels[gorder]

    shard_labels = [l_sorted[c * N_LOC : (c + 1) * N_LOC] for c in range(N_CORES)]
    tile_u = [
        [np.unique(ls[t * P : (t + 1) * P]) for t in range(T)]
        for ls in shard_labels
    ]
    u_max = max(len(u) for us in tile_u for u in us)
    u_rows = min(P, -(-u_max // 8) * 8)
    while P % u_rows:
        u_rows += 8
    tpg = P // u_rows

    if u_rows not in _nc_cache:
        _nc_cache[u_rows] = _build(u_rows)
    nc, G, n_xsq = _nc_cache[u_rows]

    in_maps = []
    for c in range(N_CORES):
        m = _prep_core(shard_labels[c], tile_u[c], w, u_rows, tpg, G)
        m["b0"], m["b1"], m["b2"] = _pack_x(x_sorted[c * N_LOC : (c + 1) * N_LOC])
        in_maps.append(m)

    # The axon-tunneled device occasionally starts in a wedged state left by
    # a previous process and recovers after a short wait; retry around it.
    last_exc = None
    for attempt in range(5):
        try:
            res = bass_utils.run_bass_kernel_spmd(
                nc, in_maps, core_ids=list(range(N_CORES))
            )
            break
        except Exception as exc:  # noqa: BLE001 — device transients
            last_exc = exc
            import time as _time

            _time.sleep(20 * (attempt + 1))
    else:
        raise last_exc
    LAST_RESULTS = res

    total = 0.0
    for c in range(N_CORES):
        acc = res.results[c]["acc"].astype(np.float64)
        total += acc[:, 0:n_xsq].sum() - 2.0 * acc[:, n_xsq : n_xsq + G].sum()
    return np.float32(total / N)


# revision 14
# speedup vs baseline: 1.2769x; 1.1727x over previous
"""CategoryConsistencyLoss kernel for 8 trn2 NeuronCores.

loss = mean_i clip(||x_i - w_{labels_i}||^2, 1e-12, 1e12)

The reference materializes the full [N, C] squared-distance matrix and
gathers the label-indexed entries. Two observations collapse the work:

1. Only the N label-indexed entries matter -> O(N*D), not O(N*C*D).
2. The output is a SCALAR mean, and the clip never binds for data in this
   regime (row distances concentrate around 2*D = 4096, far inside
   [1e-12, 1e12]), so per-row distances are never needed:

       loss * N = sum(x*x) + sum_c cnt_c*||w_c||^2 - 2*sum_c <S_c, w_c>

   where S_c = sum of x rows with label c. S is computed on the
   TensorEngine as sel^T @ x (sel built on-device from label codes), and
   the cnt*||w||^2 term is folded into the same PSUM accumulation via one
   extra matmul with lhsT = diag(-0.5*cnt):

       loss * N = sum(x*x) - 2*sum<S', wt>,  S' = sel^T @ x - 0.5*cnt (.) wt

Rows are sorted by label on the host so each 128-row tile spans <=16
distinct classes; per-tile class windows pack into G=2 combined [128, D]
weight tables (duplicate classes across tiles are harmless: the per-slot
dot/cnt sums still total correctly).

Performance structure (per core):
- x streams as fp8-e4m3 (4.2 MB vs 16.8 f32; quantization bias ~8e-4,
  gate is 2e-2) into one contiguous [128, T*D] SBUF buffer, few large
  DMAs (each sync-queue trigger costs ~0.6us serially, so fewer is
  faster), front chunks small so compute starts early.
- sel^T @ x runs as fp8 DoubleRow matmuls: two 128-row tiles (K=256)
  per instruction at 0.5 cycles/column, halving both matmul streaming
  time and LDWEIGHTS count vs plain per-tile matmuls.
- sum(x*x) splits between ACT (square+accumulate) and DVE
  (scalar_tensor_tensor fused square+reduce) using 3D access patterns
  over the single buffer: ACT takes columns [0, CA) of every tile, DVE
  the rest, in multi-tile-span instructions (front/tail spans small).
- iota ships from the host as a [1, T*P] row DMA-broadcast to all
  partitions (a GPSIMD iota takes ~3.7us and gated the sel build).
- All 16 sel one-hot matrices build in one DVE is_equal op.

Sharding: data-parallel over N across the 8 cores. Each core returns
[128, n_ops+G] partial sums; the host does the final (tiny) reduction.
"""

import numpy as np
import ml_dtypes

import concourse.bacc as bacc
import concourse.mybir as mybir
import concourse.tile as tile
from concourse import bass_utils

N, C, D = 16384, 1000, 2048
N_CORES = 8
N_LOC = N // N_CORES  # 2048 rows per core
P = 128               # SBUF partitions
T = N_LOC // P        # 16 tiles per core

CA = 1328             # ACT's column share per tile; DVE gets D - CA
XCHUNK = [2, 2, 4, 4, 2, 1, 1]   # tiles per xsq instruction
B_DMA = [1, 1, 2, 4, 4, 4]       # tiles per x dma_start

_nc_cache = {}
LAST_RESULTS = None  # BassKernelResults of the most recent run (for profiling)

F8 = mybir.dt.float8e4
F8_NP = ml_dtypes.float8_e4m3


def _spans(n_tiles, chunks):
    out, t = [], 0
    for c in chunks:
        if t >= n_tiles:
            break
        c = min(c, n_tiles - t)
        out.append((t, t + c))
        t += c
    while t < n_tiles:
        out.append((t, t + 1))
        t += 1
    return out


def _build(u_rows):
    tpg = P // u_rows          # tiles per group
    G = -(-T // tpg)           # number of groups
    pair = tpg >= 2            # DoubleRow needs tile pairs within a group
    nc = bacc.Bacc("TRN2", target_bir_lowering=False, debug=False)
    f32 = mybir.dt.float32
    bf16 = mybir.dt.bfloat16
    b_d = nc.dram_tensor("b", [P, T * D], F8, kind="ExternalInput")
    wt_d = nc.dram_tensor("wt", [G * P, D], F8, kind="ExternalInput")
    io_d = nc.dram_tensor("io", [1, T * P], f32, kind="ExternalInput")
    e2_d = nc.dram_tensor("e2", [P, T], f32, kind="ExternalInput")
    dc_d = nc.dram_tensor("dc", [G * P, P], bf16, kind="ExternalInput")
    n_ops = 2 * len(_spans(T, XCHUNK))
    out_d = nc.dram_tensor("acc", [P, n_ops + G], f32, kind="ExternalOutput")

    with tile.TileContext(nc) as tc:
        with (
            tc.tile_pool(name="big", bufs=1) as bpool,
            tc.tile_pool(name="psum", bufs=1, space="PSUM") as pspool,
            tc.tile_pool(name="small", bufs=1) as spool,
        ):
            b = bpool.tile([P, T * D], F8)
            e2_sb = spool.tile([P, T], f32)
            iota_sb = spool.tile([P, T * P], f32)
            sels = spool.tile([P, T * P], F8)
            acc = spool.tile([P, n_ops + G], f32)
            scr_a = spool.tile([P, 4 * CA], bf16)
            scr_d = spool.tile([P, 4 * (D - CA)], bf16)
            scr_big = spool.tile([P, D], bf16)
            wt_sb = [None] * G
            dc_sb = [None] * G
            S = [None] * G

            def load_group(g):
                wt_sb[g] = spool.tile([P, D], F8, tag=f"wt{g}", name=f"wt{g}")
                nc.sync.dma_start(
                    out=wt_sb[g][:], in_=wt_d.ap()[g * P : (g + 1) * P, :]
                )
                dc_sb[g] = spool.tile([P, P], bf16, tag=f"dc{g}", name=f"dc{g}")
                nc.sync.dma_start(
                    out=dc_sb[g][:], in_=dc_d.ap()[g * P : (g + 1) * P, :]
                )

            bc = _spans(T, B_DMA)

            def dma_b(ci):
                t0, t1 = bc[ci]
                nc.sync.dma_start(
                    out=b[:, t0 * D : t1 * D], in_=b_d.ap()[:, t0 * D : t1 * D]
                )

            nc.sync.dma_start(
                out=iota_sb[:], in_=io_d.ap().to_broadcast([P, T * P])
            )
            nc.sync.dma_start(out=e2_sb[:], in_=e2_d.ap()[:])
            dma_b(0)
            dma_b(1)
            dma_b(2)
            load_group(0)
            dma_b(3)
            if G > 1:
                load_group(1)
            dma_b(4)
            dma_b(5)
            for g in range(2, G):
                load_group(g)  # u_rows > 16 generality; unused when G == 2

            # sel[row, t*P + slot] = (e2[row, t] == slot): all 16 one-hot
            # selection matrices in one is_equal op, exact 0/1 in fp8.
            nc.vector.tensor_tensor(
                out=sels[:].rearrange("p (t s) -> p t s", s=P),
                in0=iota_sb[:].rearrange("p (t s) -> p t s", s=P),
                in1=e2_sb[:].unsqueeze(2).to_broadcast([P, T, P]),
                op=mybir.AluOpType.is_equal,
            )

            # sel^T @ x in fp8 DoubleRow pairs, accumulated per group in
            # PSUM; then the diag(-0.5*cnt) @ wt fold and the DVE drain.
            step = 2 if pair else 1
            for t in range(0, T, step):
                g = t // tpg
                if t % tpg == 0:
                    S[g] = pspool.tile([P, D], f32, tag=f"S{g % 2}", name=f"S{g}")
                start = t % tpg == 0
                if pair:
                    lhsT = sels[:, t * P : (t + 2) * P].rearrange(
                        "p (s m) -> p s m", s=2
                    )
                    rhs2 = b[:, t * D : (t + 2) * D].rearrange(
                        "p (s c) -> p s c", s=2
                    )
                    for q in range(4):
                        nc.tensor.matmul(
                            out=S[g][:, q * 512 : (q + 1) * 512],
                            lhsT=lhsT,
                            rhs=rhs2[:, :, q * 512 : (q + 1) * 512],
                            start=start,
                            stop=False,
                            perf_mode=mybir.MatmulPerfMode.DoubleRow,
                        )
                else:
                    for q in range(4):
                        nc.tensor.matmul(
                            out=S[g][:, q * 512 : (q + 1) * 512],
                            lhsT=sels[:, t * P : (t + 1) * P],
                            rhs=b[:, t * D + q * 512 : t * D + (q + 1) * 512],
                            start=start,
                            stop=False,
                        )
                if (t + step - 1) % tpg == tpg - 1 or t + step >= T:
                    for q in range(4):
                        nc.tensor.matmul(
                            out=S[g][:, q * 512 : (q + 1) * 512],
                            lhsT=dc_sb[g][:],
                            rhs=wt_sb[g][:, q * 512 : (q + 1) * 512],
                            start=False,
                            stop=True,
                        )
                    nc.vector.scalar_tensor_tensor(
                        out=scr_big[:],
                        in0=S[g][:],
                        scalar=1.0,
                        in1=wt_sb[g][:],
                        op0=mybir.AluOpType.mult,
                        op1=mybir.AluOpType.mult,
                        accum_out=acc[:, n_ops + g : n_ops + g + 1],
                    )

            # sum(x*x): ACT takes columns [0, CA) of every tile, DVE takes
            # [CA, D), via strided 3D access patterns over the one buffer.
            def col_view(t0, t1, c0, c1):
                return b[:, t0 * D : t1 * D].rearrange("p (t c) -> p t c", c=D)[
                    :, :, c0:c1
                ]

            ai = 0
            for t0, t1 in _spans(T, XCHUNK):
                k = t1 - t0
                nc.scalar.activation(
                    out=scr_a[:, 0 : k * CA].rearrange("p (t c) -> p t c", c=CA),
                    in_=col_view(t0, t1, 0, CA),
                    func=mybir.ActivationFunctionType.Square,
                    accum_out=acc[:, ai : ai + 1],
                )
                ai += 1
            for t0, t1 in _spans(T, XCHUNK):
                k = t1 - t0
                nc.vector.scalar_tensor_tensor(
                    out=scr_d[:, 0 : k * (D - CA)].rearrange(
                        "p (t c) -> p t c", c=D - CA
                    ),
                    in0=col_view(t0, t1, CA, D),
                    scalar=1.0,
                    in1=col_view(t0, t1, CA, D),
                    op0=mybir.AluOpType.mult,
                    op1=mybir.AluOpType.mult,
                    accum_out=acc[:, ai : ai + 1],
                )
                ai += 1
            assert ai == n_ops

            nc.sync.dma_start(out=out_d.ap()[:], in_=acc[:])
    nc.compile()
    return nc, G, n_ops


def _prep_core(ls_c, tile_u, w, u_rows, tpg, G):
    """Per-core host-side packing: weight tables, codes, count diagonals."""
    wt = np.zeros((G * P, D), dtype=np.float32)
    e2 = np.zeros((P, T), dtype=np.float32)
    cnt = np.zeros((G, P), dtype=np.float64)
    for t in range(T):
        gu = tile_u[t]
        g = t // tpg
        slot = (t % tpg) * u_rows
        wt[g * P + slot : g * P + slot + len(gu)] = w[gu]
        codes = slot + np.searchsorted(gu, ls_c[t * P : (t + 1) * P])
        e2[:, t] = codes
        cnt[g] += np.bincount(codes.astype(np.int64), minlength=P)
    dc = np.zeros((G * P, P), dtype=np.float32)
    for g in range(G):
        dc[g * P : (g + 1) * P][np.arange(P), np.arange(P)] = -0.5 * cnt[g]
    return {
        "wt": wt.astype(F8_NP),
        "e2": e2,
        "dc": dc.astype(ml_dtypes.bfloat16),
    }


def kernel(x, labels, weightcenters):
    global LAST_RESULTS
    x = np.asarray(x, dtype=np.float32)
    labels = np.asarray(labels, dtype=np.int32)
    w = np.asarray(weightcenters, dtype=np.float32)

    # Global sort by label so each 128-row tile spans few classes.
    gorder = np.argsort(labels, kind="stable")
    x_sorted = np.ascontiguousarray(x[gorder]).astype(F8_NP)
    l_sorted = labels[gorder]

    shard_labels = [l_sorted[c * N_LOC : (c + 1) * N_LOC] for c in range(N_CORES)]
    tile_u = [
        [np.unique(ls[t * P : (t + 1) * P]) for t in range(T)]
        for ls in shard_labels
    ]
    u_max = max(len(u) for us in tile_u for u in us)
    u_rows = min(P, -(-u_max // 8) * 8)
    while P % u_rows:
        u_rows += 8
    tpg = P // u_rows

    if u_rows not in _nc_cache:
        _nc_cache[u_rows] = _build(u_rows)
    nc, G, n_ops = _nc_cache[u_rows]

    iota = np.tile(np.arange(P, dtype=np.float32), T)[None, :]
    in_maps = []
    for c in range(N_CORES):
        m = _prep_core(shard_labels[c], tile_u[c], w, u_rows, tpg, G)
        xs = x_sorted[c * N_LOC : (c + 1) * N_LOC].reshape(T, P, D)
        m["b"] = np.ascontiguousarray(xs.transpose(1, 0, 2)).reshape(P, T * D)
        m["io"] = iota
        in_maps.append(m)

    # The axon-tunneled device occasionally starts in a wedged state left by
    # a previous process and recovers after a short wait; retry around it.
    last_exc = None
    for attempt in range(5):
        try:
            res = bass_utils.run_bass_kernel_spmd(
                nc, in_maps, core_ids=list(range(N_CORES))
            )
            break
        except Exception as exc:  # noqa: BLE001 — device transients
            last_exc = exc
            import time as _time

            _time.sleep(20 * (attempt + 1))
    else:
        raise last_exc
    LAST_RESULTS = res

    total = 0.0
    for c in range(N_CORES):
        acc = res.results[c]["acc"].astype(np.float64)
        total += acc[:, 0:n_ops].sum() - 2.0 * acc[:, n_ops : n_ops + G].sum()
    return np.float32(total / N)
